# revision 100
# baseline (speedup 1.0000x reference)
"""Trainium2 Bass kernel for multi-head attention (B=1, N=4096, C=768, H=12, D=64).

Sharding: tensor-parallel over heads across 8 cores. Core c (pair k=c//2):
  even c: head A = 3k   (all queries),  head B = 3k+1 (local queries 0-2047)
  odd  c: head A = 3k+2 (all queries),  head B = 3k+1 (local queries 0-2047,
          with x^T columns rotated by 2048 so these are global 2048-4095)
Key/value sums are permutation invariant, so the rotation only permutes rows
of the per-core output, which the host un-permutes before summing partials.

Matmuls: projections/scores/out-proj bf16-in fp32-PSUM-out; AV runs fp8e4
DoubleRow (exp slab + V stored e4m3, adjacent key-block pairs folded into the
DR ko slots) at 0.5 PE cycles/col.  Per core:
  - QK projection emits [qA|qB] and [kA|kB] on partition halves; evacuations
    are paired (two 512-col chunks per copy).
  - Scores S^T[m, q] per 4-mb quad [128, 4, 256] fp32 (2 PSUM banks), exp'd
    in one 1024-wide instruction writing the e4m3 slab: ScalarE true exp or
    DVE Schraudolph (int8 bitcast e4m3 exp2 trick), chosen by a greedy
    Act/DVE load balancer that also places all PSUM evacuation copies.
  - AV accumulates O[q, 65] per 128-query block via DoubleRow mb-pairs (ones
    column of V gives row sums in col 64).  Finalize: one batched reciprocal
    per unit + per-j fused scale to bf16.
  - Per query block, O^T for heads A and B via two PE transposes into one
    stacked [128, 128] PSUM tile, then a K=128 output projection.
PSUM: 3 score-quad slots (6 banks) + 2 shared acc/finalize banks = 8.
Two ordered work queues (paced projection chunks; lagged AV + finalize
steps) drain between score quads to keep every engine busy.
"""

import sys
from collections import deque

for _p in ("/opt/trn_rl_repo",):
    if _p not in sys.path:
        sys.path.insert(0, _p)

import numpy as np
import ml_dtypes

import concourse.bass as bass  # noqa: F401
import concourse.mybir as mybir
from concourse import bacc, tile
from concourse.bass_utils import run_bass_kernel_spmd

F32 = mybir.dt.float32
BF16 = mybir.dt.bfloat16
F8E4 = mybir.dt.float8e4
I16 = mybir.dt.int16
I8 = mybir.dt.int8
AF = mybir.ActivationFunctionType
ALU = mybir.AluOpType
PM = mybir.MatmulPerfMode
NPBF16 = ml_dtypes.bfloat16

N = 4096
C = 768
D = 64
NB = 8          # 512-column blocks of n
NMB = 32        # 128-row m blocks
NQUAD = 8       # 4-mb quads per unit
UNITS_A = 16    # 256-query units, head A
UNITS_B = 8     # head B (half the queries)
SCALE = D ** -0.5

# Schraudolph exp2 constants (int bitcast tricks, validated on HW)
# f32->int8 convert on DVE rounds-to-nearest (HW-verified), so no +0.5
SCH8_A = 8.0 / float(np.log(2.0))
SCH8_B = 8.0 * (7.0 - 0.0433)
SCH16_A = 128.0 / float(np.log(2.0))
SCH16_B = 128.0 * (127.0 - 0.0433)
import os
AV_BF16 = bool(os.environ.get("AV_BF16"))
SLAB_DT = None  # set at import below
POP_CAP = 8
AV_LAG = 4

# engine cost estimates (ns) for the greedy Act/DVE balancer
def _act_cost(el):
    return el * 0.8333 + 300.0


def _dve_cost(el):
    return el * 1.0417 + 195.0


_NC = None


def _emit(nc, tc, io, ctx):
    xt, wq2, wk2, wv2, wp, ident, y_out = (
        io["xt"], io["wq2"], io["wk2"], io["wv2"], io["wp"], io["ident"],
        io["y"])

    sing = ctx.enter_context(tc.tile_pool(name="sing", bufs=1))
    spool = ctx.enter_context(tc.tile_pool(name="spool", bufs=3, space="PSUM"))
    apool = ctx.enter_context(tc.tile_pool(name="apool", bufs=2, space="PSUM"))
    ppool = ctx.enter_context(tc.tile_pool(name="ppool", bufs=3))
    opool = ctx.enter_context(tc.tile_pool(name="opool", bufs=6))
    tpool = ctx.enter_context(tc.tile_pool(name="tpool", bufs=4))
    ygp = ctx.enter_context(tc.tile_pool(name="ygp", bufs=8))
    rpool = ctx.enter_context(tc.tile_pool(name="rpool", bufs=4))

    # ---- greedy Act/DVE balance state ----
    busy = {"A": 0.0, "D": 0.0}
    expctr = [0]

    def pick(el):
        """Pick engine for a copy/evac of `el` free elements."""
        ca = busy["A"] + _act_cost(el)
        cd = busy["D"] + _dve_cost(el)
        if ca <= cd:
            busy["A"] = ca
            return "A"
        busy["D"] = cd
        return "D"

    def copy(dst, src):
        el = src.free_size()
        if pick(el) == "A":
            nc.scalar.copy(dst, src)
        else:
            nc.vector.tensor_copy(dst, src)

    def scopy(dst, src, factor):
        el = src.free_size()
        if pick(el) == "A":
            nc.scalar.activation(out=dst, in_=src, func=AF.Copy, scale=factor)
        else:
            nc.vector.tensor_scalar(out=dst, in0=src, scalar1=factor,
                                    scalar2=None, op0=ALU.mult)

    # ---------------- PE warm-up during initial DMA wait ----------------
    scratch = sing.tile([128, 128], BF16, name="scratch", tag="scratch")
    nc.vector.memset(scratch, 0.25)
    busy["D"] += 330.0
    wu_ps = spool.tile([128, 128], F32, name="wu_ps", tag="sq")
    for _ in range(24):
        nc.tensor.matmul(wu_ps, lhsT=scratch, rhs=scratch,
                         start=True, stop=True)

    # ---- input DMAs, ordered so the first proj chunks unblock earliest ----
    w_sb = {}
    # xt_sb[:, cc, 0, :] = x_hi, [:, cc, 1, :] = x_lo (e4m3 hi/lo split)
    xt_sb = sing.tile([128, 6, 2, N], F8E4, name="xt_sb", tag="xt_sb")

    def wdma(nm, src):
        # [:, cc, 0:2, :] = W_hi duplicated (DR ko pair), [:, cc, 2, :] = W_lo
        t = sing.tile([128, 6, 3, 128], F8E4, name=f"{nm}_sb", tag=f"{nm}_sb")
        nc.sync.dma_start(out=t, in_=src)
        w_sb[nm] = t

    def xdma(nb):
        nc.sync.dma_start(out=xt_sb[:, :, :, nb * 512:(nb + 1) * 512],
                          in_=xt[:, :, :, nb * 512:(nb + 1) * 512])

    wdma("wk2", wk2)
    nc.sync.dma_start(out=xt_sb[:, 0:2, :, 0:512], in_=xt[:, 0:2, :, 0:512])
    nc.sync.dma_start(out=xt_sb[:, 2:4, :, 0:512], in_=xt[:, 2:4, :, 0:512])
    nc.sync.dma_start(out=xt_sb[:, 4:6, :, 0:512], in_=xt[:, 4:6, :, 0:512])
    wdma("wq2", wq2)
    wdma("wv2", wv2)
    nc.sync.dma_start(out=xt_sb[:, 0:3, :, 512:1024], in_=xt[:, 0:3, :, 512:1024])
    nc.sync.dma_start(out=xt_sb[:, 3:6, :, 512:1024], in_=xt[:, 3:6, :, 512:1024])
    for nb in range(2, NB):
        xdma(nb)
    wp_sb = sing.tile([128, C], BF16, name="wp_sb", tag="wp_sb")
    nc.sync.dma_start(out=wp_sb, in_=wp)
    id_sb = sing.tile([128, 128], BF16, name="id_sb", tag="id_sb")
    nc.sync.dma_start(out=id_sb, in_=ident)

    qab = sing.tile([128, N], BF16, name="qab", tag="qab")
    kab = sing.tile([128, N], BF16, name="kab", tag="kab")
    # vsl per mb: head A v at 0:64, ones col 64; head B at 80:144, ones 144
    vdt = BF16 if AV_BF16 else F8E4
    vsl = sing.tile([128, NMB, 160], vdt, name="vsl", tag="vsl")
    nc.vector.memset(vsl[:, :, 64], 1.0)
    nc.vector.memset(vsl[:, :, 144], 1.0)
    busy["D"] += 660.0

    # ---------------- projection chunks (paired) ----------------
    def qk_mms(out_ps, w, nb):
        """W_hi·(x_hi+x_lo) via 6 DR mms + W_lo·x_hi via 3 cc-paired DR mms."""
        lo, hi = nb * 512, (nb + 1) * 512
        for cc in range(6):
            nc.tensor.matmul(
                out_ps, lhsT=w[:, cc, 0:2, :], rhs=xt_sb[:, cc, :, lo:hi],
                start=(cc == 0), stop=False, perf_mode=PM.DoubleRow,
                skip_group_check=True)
        for cc in range(0, 6, 2):
            nc.tensor.matmul(
                out_ps, lhsT=w[:, cc:cc + 2, 2, :],
                rhs=xt_sb[:, cc:cc + 2, 0, lo:hi],
                start=False, stop=(cc == 4), perf_mode=PM.DoubleRow,
                skip_group_check=True)

    def qk_pair(dst, w, p, factor):
        """Two 512-col chunks (nb=2p, 2p+1) -> one 1024-el scaled evac."""
        def emit():
            ps = spool.tile([128, 1024], F32, name="ps_qk", tag="sq")
            for half in range(2):
                qk_mms(ps[:, half * 512:(half + 1) * 512], w, 2 * p + half)
            scopy(dst[:, 2 * p * 512:(2 * p + 2) * 512], ps, factor)
        return emit

    def v_pair(P):
        """Four key mbs (4P..4P+3) -> one [128, 4, 2, 64] evacuation."""
        def emit():
            ps = spool.tile([128, 512], F32, name="ps_v", tag="sq")
            wv = w_sb["wv2"]
            for half in range(4):
                mb = 4 * P + half
                lo, hi = mb * 128, (mb + 1) * 128
                out = ps[:, half * 128:(half + 1) * 128]
                for cc in range(6):
                    nc.tensor.matmul(
                        out, lhsT=xt_sb[:, cc, :, lo:hi],
                        rhs=wv[:, cc, 0:2, :],
                        start=(cc == 0), stop=False, perf_mode=PM.DoubleRow,
                        skip_group_check=True)
                for cc in range(0, 6, 2):
                    nc.tensor.matmul(
                        out, lhsT=xt_sb[:, cc:cc + 2, 0, lo:hi],
                        rhs=wv[:, cc:cc + 2, 2, :],
                        start=False, stop=(cc == 4), perf_mode=PM.DoubleRow,
                        skip_group_check=True)
            dst = vsl[:, 4 * P:4 * P + 4, :].rearrange(
                "p m (two c) -> p m two c", two=2)[:, :, :, 0:64]
            src = ps.rearrange("p (m two c) -> p m two c", m=4, two=2)
            scopy(dst, src, 1.0 / 32.0)
        return emit

    def qk_single(dst, w, nb, factor):
        """One 512-col chunk with its own scaled evac (startup path)."""
        def emit():
            ps = spool.tile([128, 512], F32, name="ps_qk1", tag="sq")
            qk_mms(ps, w, nb)
            scopy(dst[:, nb * 512:(nb + 1) * 512], ps, factor)
        return emit

    # upfront: only what quad 0 needs (kab/qab cols 0:512)
    qk_single(kab, w_sb["wk2"], 0, 1.0 / 32.0)()
    qk_single(qab, w_sb["wq2"], 0, SCALE / 32.0)()

    # proj queue: K eager, V/Q just-in-time so attention (and exp) start at
    # quad 0 and proj matmuls fill PE slack instead of delaying exp.
    items = [
        (0, v_pair(0)),
        (0, qk_single(kab, w_sb["wk2"], 1, 1.0 / 32.0)),
        (1, qk_single(qab, w_sb["wq2"], 1, SCALE / 32.0)),
    ]
    for p in range(1, 4):
        # scores(A0, quads 2p,2p+1) need K pair p just in time
        items.append((max(0, 2 * p - 2), qk_pair(kab, w_sb["wk2"], p, 1.0 / 32.0)))
    qmins = {1: 52, 2: 116, 3: 148}
    for p in range(1, 4):
        items.append((qmins[p], qk_pair(qab, w_sb["wq2"], p, SCALE / 32.0)))
    for P in range(1, NMB // 4):
        # AV(A0, quad t) pops at gq t+AV_LAG and reads V mbs 4t..4t+3
        items.append((P + 2, v_pair(P)))
    proj = deque(sorted(items, key=lambda x: x[0]))

    work = deque()   # AV/finalize items: (min_gq, emit_fn), strict order

    # ---------------- attention units ----------------
    units = []
    for u in range(UNITS_B):
        units.append((0, u))
        units.append((1, u))
    for u in range(UNITS_B, UNITS_A):
        units.append((0, u))

    pair_state = {}

    def make_av(st, t):
        def emit():
            if st["acc"] is None:
                st["acc"] = [apool.tile([128, 65], F32, name="acc", tag="acc")
                             for _ in range(2)]
            h, slab = st["h"], st["slab"]
            if AV_BF16:
                for i in range(4):
                    mb = 4 * t + i
                    for j in range(2):
                        nc.tensor.matmul(
                            st["acc"][j],
                            lhsT=slab[:, mb, j * 128:(j + 1) * 128],
                            rhs=vsl[:, mb, h * 80:h * 80 + 65],
                            start=(mb == 0), stop=(mb == NMB - 1),
                            skip_group_check=True)
                return
            for i in range(2):
                mb = 4 * t + 2 * i
                for j in range(2):
                    nc.tensor.matmul(
                        st["acc"][j],
                        lhsT=slab[:, mb:mb + 2, j * 128:(j + 1) * 128],
                        rhs=vsl[:, mb:mb + 2, h * 80:h * 80 + 65],
                        start=(mb == 0), stop=(mb == NMB - 2),
                        perf_mode=PM.DoubleRow,
                        skip_group_check=True)
        return emit

    def make_recip(st):
        def emit():
            rinv = rpool.tile([128, 2], F32, name="rinv", tag="rinv")
            for j in range(2):
                nc.vector.reciprocal(rinv[:, j:j + 1],
                                     st["acc"][j][:, 64:65])
            busy["D"] += 400.0
            st["rinv"] = rinv
        return emit

    def make_stepA(st, j):
        def emit():
            osb = opool.tile([128, 64], BF16, name="osb", tag="osb")
            el = 64
            if pick(el) == "A":
                nc.scalar.activation(
                    out=osb, in_=st["acc"][j][:, 0:64], func=AF.Copy,
                    scale=st["rinv"][:, j:j + 1])
            else:
                nc.vector.tensor_scalar(
                    out=osb, in0=st["acc"][j][:, 0:64],
                    scalar1=st["rinv"][:, j:j + 1], scalar2=None,
                    op0=ALU.mult)
            st["osb"].append(osb)
        return emit

    def make_stepT(stA, stB, u, j, phase):
        def emit():
            if phase == 0:
                tps = apool.tile([128, 128], BF16, name="tps", tag="acc")
                nc.tensor.transpose(tps[0:64, :], stA["osb"][j], id_sb)
                if stB is not None:
                    nc.tensor.transpose(tps[64:128, :], stB["osb"][j], id_sb)
                wl = 128 if stB is not None else 64
                tsb = tpool.tile([128, 128], BF16, name="tsb", tag="tsb")
                copy(tsb[0:wl, :], tps[0:wl, :])
                stA["tsb"][j] = (tsb, wl)
                yp = apool.tile([128, 512], F32, name="yp", tag="acc")
                nc.tensor.matmul(yp, lhsT=tsb[0:wl, :], rhs=wp_sb[0:wl, 0:512],
                                 start=True, stop=True)
                ysb = ygp.tile([128, C], BF16, name="ysb", tag="ysb")
                copy(ysb[:, 0:512], yp)
                stA["ysb"][j] = ysb
            else:
                tsb, wl = stA["tsb"][j]
                yp = apool.tile([128, 256], F32, name="yp2", tag="acc")
                nc.tensor.matmul(yp, lhsT=tsb[0:wl, :],
                                 rhs=wp_sb[0:wl, 512:C],
                                 start=True, stop=True)
                ysb = stA["ysb"][j]
                copy(ysb[:, 512:C], yp)
                row = (2 * u + j) * 128
                nc.sync.dma_start(out=y_out[row:row + 128, :], in_=ysb)
        return emit

    gq = 0
    nunits = len(units)
    for ui, (h, u) in enumerate(units):
        last = ui >= nunits - 5
        av_lag = 2 if last else AV_LAG
        st = {"h": h, "u": u, "slab": ppool.tile(
            [128, NMB, 256], BF16 if AV_BF16 else F8E4, name="pslab",
            tag="pslab"),
            "acc": None, "rinv": None, "osb": [], "tsb": {}, "ysb": {}}
        pair_state[(h, u)] = st
        base = h * 64
        qlo = u * 256
        for t in range(NQUAD):
            quad = spool.tile([128, 4, 256], F32, name="quad", tag="sq")
            for i in range(4):
                mb = 4 * t + i
                nc.tensor.matmul(
                    quad[:, i, :],
                    lhsT=kab[base:base + 64, mb * 128:(mb + 1) * 128],
                    rhs=qab[base:base + 64, qlo:qlo + 256],
                    start=True, stop=True)
            dst = st["slab"][:, 4 * t:4 * t + 4, :]
            el = 1024
            i8 = expctr[0] % 15
            expctr[0] += 1
            eng = "A" if i8 in (0, 2, 4, 6, 8, 10, 12, 13) else "D"
            busy[eng] += _act_cost(el) if eng == "A" else _dve_cost(el)
            if os.environ.get("ALL_ACT_EXP") or eng == "A":
                nc.scalar.activation(out=dst, in_=quad, func=AF.Exp)
            elif AV_BF16:
                nc.vector.tensor_scalar(
                    out=dst.bitcast(I16), in0=quad, scalar1=SCH16_A,
                    scalar2=SCH16_B, op0=ALU.mult, op1=ALU.add)
            else:
                nc.vector.tensor_scalar(
                    out=dst.bitcast(I8), in0=quad, scalar1=SCH8_A,
                    scalar2=SCH8_B, op0=ALU.mult, op1=ALU.add)
            work.append((gq + av_lag, make_av(st, t)))
            if t == NQUAD - 1:
                sl = 2 if last else 0
                work.append((gq + 3 - sl, make_recip(st)))
                work.append((gq + 3 - sl, make_stepA(st, 0)))
                work.append((gq + 4 - sl, make_stepA(st, 1)))
                if h == 1:
                    stA = pair_state[(0, u)]
                    for j in range(2):
                        work.append((gq + 5 + 4 * j, make_stepT(stA, st, u, j, 0)))
                        work.append((gq + 7 + 4 * j, make_stepT(stA, st, u, j, 1)))
                elif u >= UNITS_B:
                    if last:
                        for j in range(2):
                            work.append((gq + 1 + 2 * j,
                                         make_stepT(st, None, u, j, 0)))
                            work.append((gq + 2 + 2 * j,
                                         make_stepT(st, None, u, j, 1)))
                    else:
                        for j in range(2):
                            work.append((gq + 5 + 4 * j,
                                         make_stepT(st, None, u, j, 0)))
                            work.append((gq + 7 + 4 * j,
                                         make_stepT(st, None, u, j, 1)))
            npop = 0
            while npop < POP_CAP:
                if proj and proj[0][0] <= gq:
                    proj.popleft()[1]()
                elif work and work[0][0] <= gq:
                    work.popleft()[1]()
                else:
                    break
                npop += 1
            gq += 1
    while proj:
        proj.popleft()[1]()
    while work:
        work.popleft()[1]()


def _build():
    nc = bacc.Bacc("TRN2", debug=False, enable_asserts=False, num_devices=8)
    io = {
        "xt": nc.dram_tensor("xt", [128, 6, 2, N], F8E4,
                             kind="ExternalInput").ap(),
        "wq2": nc.dram_tensor("wq2", [128, 6, 3, 128], F8E4,
                              kind="ExternalInput").ap(),
        "wk2": nc.dram_tensor("wk2", [128, 6, 3, 128], F8E4,
                              kind="ExternalInput").ap(),
        "wv2": nc.dram_tensor("wv2", [128, 6, 3, 128], F8E4,
                              kind="ExternalInput").ap(),
        "wp": nc.dram_tensor("wp", [128, C], BF16, kind="ExternalInput").ap(),
        "ident": nc.dram_tensor("ident", [128, 128], BF16,
                                kind="ExternalInput").ap(),
        "y": nc.dram_tensor("y", [N, C], BF16, kind="ExternalOutput").ap(),
    }
    from contextlib import ExitStack
    with tile.TileContext(nc) as tc, ExitStack() as ctx:
        _emit(nc, tc, io, ctx)
    nc.compile()
    return nc


def _get_nc():
    global _NC
    if _NC is None:
        _NC = _build()
    return _NC


NPE4 = ml_dtypes.float8_e4m3


def _hilo(a):
    """Split fp32 array into (hi, lo) e4m3 parts: hi + lo ~= a."""
    hi = a.astype(NPE4)
    lo = (a - hi.astype(np.float32)).astype(NPE4)
    return hi, lo


def _in_maps(x, W_qkv, W_proj):
    xT = np.ascontiguousarray(x[0].T.astype(np.float32))  # [768, 4096]
    rot = np.concatenate([np.arange(2048, N), np.arange(0, 2048)])
    ident = np.eye(128, dtype=np.float32)

    def head_rows(h, off):
        return W_qkv[off + h * D:off + (h + 1) * D, :]  # [64, 768]

    maps = []
    for c in range(8):
        k = c // 2
        hA = 3 * k if c % 2 == 0 else 3 * k + 2
        hB = 3 * k + 1
        cols = np.arange(N) if c % 2 == 0 else rot
        xt = xT[:, cols].reshape(6, 128, N).transpose(1, 0, 2)  # [128,6,N]
        xh, xl = _hilo(np.ascontiguousarray(xt))
        xt8 = np.stack([xh, xl], axis=2)          # [128, 6, 2, N]

        def wtile(off):
            wA = head_rows(hA, off) * 32.0         # [64, 768]
            wB = head_rows(hB, off) * 32.0
            w = np.concatenate([wA, wB], axis=0)   # [128, 768] rows=out d
            # lhsT layout [p, cc, d]: value = W[d_row, cc*128+p]
            wt = np.ascontiguousarray(
                w.T.reshape(6, 128, 128).transpose(1, 0, 2))
            wh, wl = _hilo(wt)
            # [:, cc, 0:2, :] = hi duplicated (DR ko pair); [:, cc, 2, :] = lo
            return np.ascontiguousarray(np.stack([wh, wh, wl], axis=2))

        wpA = W_proj[:, hA * D:(hA + 1) * D].T     # [64, 768]
        wpB = W_proj[:, hB * D:(hB + 1) * D].T
        wp = np.concatenate([wpA, wpB], axis=0)    # [128, 768]

        maps.append({
            "xt": xt8,
            "wq2": wtile(0),
            "wk2": wtile(C),
            "wv2": wtile(2 * C),
            "wp": np.ascontiguousarray(wp).astype(NPBF16),
            "ident": ident.astype(NPBF16),
        })
    return maps


def kernel(x, xpos, W_qkv, W_proj, b_proj, _results_hook=None):
    x = np.asarray(x, dtype=np.float32)
    W_qkv = np.asarray(W_qkv, dtype=np.float32)
    W_proj = np.asarray(W_proj, dtype=np.float32)
    b_proj = np.asarray(b_proj, dtype=np.float32)

    nc = _get_nc()
    res = run_bass_kernel_spmd(nc, _in_maps(x, W_qkv, W_proj),
                               core_ids=list(range(8)))
    if _results_hook is not None:
        _results_hook(res)

    rot = np.concatenate([np.arange(2048, N), np.arange(0, 2048)])
    out = np.zeros((N, C), np.float32)
    for c in range(8):
        y = np.asarray(res.results[c]["y"]).astype(np.float32)
        gl = np.arange(N) if c % 2 == 0 else rot
        out[gl] += y
    out += b_proj[None, :]
    return out[None]
